# revision 35
# baseline (speedup 1.0000x reference)
"""Masked self-attention Trainium2 Bass kernel.

Reference computation (per batch b):
    q = x @ Wq + bq ; k = x @ Wk + bk ; v = x @ Wv + bv      # [S, A]
    scores = (q @ k.T) / sqrt(S)  with causal mask            # [S, S]
    out = softmax(scores, axis=-1) @ v                        # [S, A]

Sharding: data-parallel over batch across 8 NeuronCores (B=32 -> 4 per core),
weights replicated. No collectives.

Per-core design (mixed precision; tolerance gate is 2e-2 — measured HW
max-rel error 1.57e-2, dominated by the fp8 q/k path on few-key early rows):
  Matmul dtypes: fp32 LDWEIGHTS can't use fast-weight-load, so a 128-col
  stationary reload costs ~224ns and serializes against the matmul stream
  (measured gap 272ns for N=500 fp32r MMs vs 208ns streaming). bf16 weights
  FWL-load in ~97ns and hide completely; fp8e4+DoubleRow streams 2 rows/cyc
  (0.5 cyc/row), halving projection stream time (DR LDWEIGHTS ~135ns does
  stay mostly serial — measured 211ns/MM vs 104ns streaming).
  Stage A: DMA x[b] [S,E] fp32; ACT-convert to bf16; bf16 PE-transpose
           (1.0 cyc/row vs 1.5 fp32r), all 8 u-tiles in one full PSUM bank;
           one strided 3D-AP DVE copy drains straight to xT8 [E,S+pad]
           fp8e4 (pad keeps the u-stride 16B-aligned for DoubleRow pair
           APs); s-cols 0..128 also copied to bf16 (xT16_t0) for the v
           fixup.
  Stage B: qT/kT [A,S] via fp8e4 DoubleRow matmuls: stationary = Wq/Wk pair
           tiles (pre-scaled x512 so W ~0.02 clears fp8's 2^-6 min normal),
           moving = xT8 u-pair 3D AP; bias + 1/sqrt(S) + 1/512 folded into
           the ACT PSUM->SBUF drain -> bf16.
           v [S,A] via fp8e4 DoubleRow (stationary = xT8 pair, moving = Wv8
           pair), DVE drain with 1/512 -> bf16; no bias (softmax rows sum to
           1, bv is added to the final output); 2 ones-columns appended via
           memset. Rows 0..127 of the output are ~v verbatim, so a separate
           bf16 v (v16_t0, from xT16_t0 @ Wv16) serves q-tile 0's PV.
  Stage C: scoresT[k,q] = kT16.T @ qT16 per k-tile, causal-trimmed chunks;
           additive -1e9 mask on the diagonal block in PSUM (DVE); exp on ACT
           (PSUM->SBUF bf16). No max-subtraction: |scores| <~ 3.
  Stage D: interleaved with C per tile: out_psum = sum_t expT[t].T @ v[t] in
           two column chunks; ones-columns give softmax row-sums; DVE
           reciprocal + tensor_scalar row scale; GPSIMD adds broadcast bv;
           DMA out per 256-column half.
"""

import numpy as np
from contextlib import ExitStack

import concourse.bass as bass
import concourse.mybir as mybir
import concourse.tile as tile
from concourse import bacc
from concourse.bass_utils import run_bass_kernel_spmd
from concourse.masks import make_identity

P = 128
F32 = mybir.dt.float32
F32R = mybir.dt.float32r
BF16 = mybir.dt.bfloat16
FP8 = mybir.dt.float8e4
DR = mybir.MatmulPerfMode.DoubleRow
DRS = mybir.MatmulPerfMode.DoubleRowSwInterleave
AF = mybir.ActivationFunctionType

N_CORES = 8
B, S, E, A = 32, 1000, 1024, 512
MASK_NEG = -1.0e9
W8_SCALE = 512.0  # Wq/Wk pre-scale before fp8 cast (values ~0.02 vs 2^-6 min)
S8 = 1008  # fp8 xT row stride: 16B-aligned for DoubleRow pair APs


def _chunks(start, total, maxc):
    """Split [start, start+total) into ceil(total/maxc) near-even chunks."""
    n = max(1, -(-total // maxc))
    bounds = [start + (i * total) // n for i in range(n)]
    bounds.append(start + total)
    return [(bounds[i], bounds[i + 1] - bounds[i]) for i in range(n)]


def build(b_pc, s, e, a, reps=1):
    assert e % P == 0 and a % P == 0
    n_s = -(-s // P)
    n_e = e // P
    n_a = a // P
    inv_den = float(s) ** -0.5
    s_tiles = [(t * P, min(P, s - t * P)) for t in range(n_s)]
    h = a // 2  # PV column split: [0,h) and [h, a+2)

    n_j = e // (2 * P)
    nc = bacc.Bacc("TRN2")
    x = nc.dram_tensor("x", [b_pc, s, e], F32R, kind="ExternalInput").ap()
    # Wq/Wk arrive pre-scaled (x W8_SCALE) and pre-interleaved on the host
    # for DoubleRowSwInterleave: [j, m, K=128, 256] with per-row layout
    # [A127, B127, A126, B126, ..., A0, B0] (A = E-rows 2j*128.., B = next
    # 128, columns reversed). Wv arrives in natural [E, A] layout.
    w_dram = {
        "q": nc.dram_tensor("Wq", [n_j, a // P, P, 2 * P], F32,
                            kind="ExternalInput").ap(),
        "k": nc.dram_tensor("Wk", [n_j, a // P, P, 2 * P], F32,
                            kind="ExternalInput").ap(),
        "v": nc.dram_tensor("Wv", [e, a], F32, kind="ExternalInput").ap(),
    }
    b_dram = {
        "q": nc.dram_tensor("bq", [a], F32, kind="ExternalInput").ap(),
        "k": nc.dram_tensor("bk", [a], F32, kind="ExternalInput").ap(),
        "v": nc.dram_tensor("bv", [a], F32, kind="ExternalInput").ap(),
    }
    out = nc.dram_tensor("out", [b_pc, s, a], F32, kind="ExternalOutput").ap()

    with tile.TileContext(nc) as tc, ExitStack() as ctx:
        pool = ctx.enter_context(tc.tile_pool(name="sb", bufs=1))
        pp_tp = ctx.enter_context(tc.tile_pool(name="pp_tp", bufs=2, space="PSUM"))
        pp_proj = ctx.enter_context(tc.tile_pool(name="pp_proj", bufs=2, space="PSUM"))
        pp_score = ctx.enter_context(tc.tile_pool(name="pp_sc", bufs=2, space="PSUM"))
        pp_o1 = ctx.enter_context(tc.tile_pool(name="pp_o1", bufs=1, space="PSUM"))
        pp_o2 = ctx.enter_context(tc.tile_pool(name="pp_o2", bufs=1, space="PSUM"))

        # ---------------- constants ----------------
        ident_st = pool.tile([P, P], F32)
        make_identity(nc, ident_st)
        ident = pool.tile([P, P], BF16)
        nc.scalar.copy(ident[:], ident_st[:])

        # additive causal mask for the diagonal block:
        # keep 0 where col q >= row k (i.e. (y - x) >= 0), else fill -1e9
        amask = pool.tile([P, P], F32)
        nc.gpsimd.memset(amask, 0.0)
        nc.gpsimd.affine_select(
            out=amask, in_=amask,
            compare_op=mybir.AluOpType.is_ge,
            fill=MASK_NEG, base=0,
            pattern=[[1, P]], channel_multiplier=-1,
        )

        # ---------------- weights / biases ----------------
        # q/k/v weights: fp8e4 pair layout [P, u, a] (u-stride a bytes, 16B
        # aligned), pre-scaled by W8_SCALE. v also kept in bf16 for the
        # q-tile-0 fixup (early output rows are ~v verbatim, so the fp8 v
        # error would land on them directly).
        w8 = {}
        for nm in ("v",):
            w8_all = pool.tile([P, n_e, a], FP8, tag=f"w8_{nm}", bufs=1)
            for u in range(n_e):
                w_stage = pool.tile([P, a], F32, tag="w_stage", bufs=2)
                nc.gpsimd.dma_start(w_stage[:], w_dram[nm][u * P:(u + 1) * P, :])
                nc.scalar.mul(w8_all[:, u], w_stage[:], W8_SCALE)
            w8[nm] = w8_all
        # q/k: interleaved fp8 pair blocks for DoubleRowSwInterleave (host
        # pre-scaled by W8_SCALE; device just casts)
        w8il = {}
        for nm in ("q", "k"):
            il_all = pool.tile([P, n_j, n_a, 2 * P], FP8, tag=f"w8il_{nm}",
                               bufs=1)
            for j in range(n_j):
                for m in range(n_a):
                    w_stage = pool.tile([P, 2 * P], F32, tag="wil_stage",
                                        bufs=2)
                    nc.gpsimd.dma_start(w_stage[:], w_dram[nm][j, m])
                    nc.scalar.copy(il_all[:, j, m], w_stage[:])
            w8il[nm] = il_all
        w16v = []
        for u in range(n_e):
            w_stage = pool.tile([P, a], F32, tag="w_stage", bufs=2)
            nc.gpsimd.dma_start(w_stage[:], w_dram["v"][u * P:(u + 1) * P, :])
            w_r = pool.tile([P, a], BF16, tag="w16_v", bufs=n_e)
            nc.vector.tensor_copy(w_r[:], w_stage[:])
            w16v.append(w_r)

        bias_sb = {}
        for nm in ("q", "k"):
            b_st = pool.tile([P, n_a], F32, tag=f"b_{nm}", bufs=1)
            nc.gpsimd.dma_start(
                b_st[:], b_dram[nm].rearrange("(m p) -> p m", p=P)
            )
            bias_sb[nm] = b_st
        # pre-scale bq by 1/sqrt(S) (scores scaling folded into q)
        bqs = pool.tile([P, n_a], F32)
        nc.scalar.mul(bqs[:], bias_sb["q"][:], inv_den)
        bias_sb["q"] = bqs
        # PSUM-drain scales (undo the fp8 weight pre-scale)
        drain_scale = {"q": inv_den / W8_SCALE, "k": 1.0 / W8_SCALE}

        bv_stage = pool.tile([1, a], F32)
        nc.gpsimd.dma_start(bv_stage[:], b_dram["v"][:])
        bv_bc = pool.tile([P, a], F32)
        nc.gpsimd.partition_broadcast(bv_bc[:], bv_stage[:])

        # ---------------- per-batch pipeline ----------------
        # reps>1 wraps the whole pipeline in an on-device loop re-running the
        # same work — used only to measure device exec time (amortizes the
        # per-dispatch RPC overhead, which otherwise hides the kernel).
        rep_ctx = (tc.For_i(0, reps, 1, hint_engines=tuple(nc.engines),
                            staggered_reset=True)
                   if reps > 1 else None)
        if rep_ctx is not None:
            ctx.enter_context(rep_ctx)

        def stage_a_alloc(b):
            # stage A outputs: xT8 [E,S] fp8; a small bf16 slice (s-cols
            # 0..P) feeds the bf16 v fixup for q-tile 0
            xT16_t0 = pool.tile([P, n_e, P], BF16, tag="xT16t0", bufs=2,
                                name=f"xT16t0_{b}")
            xT8_all = pool.tile([P, n_e, S8], FP8, tag="xT8", bufs=2,
                                name=f"xT8_{b}")
            return xT8_all, xT16_t0

        def stage_a(b, xT8_all, xT16_t0, tiles):
            # ---- stage A: load x, transpose, drain to fp8 SBUF ----
            for (s0, sl) in tiles:
                x_sb = pool.tile([P, e], F32R, tag="x", bufs=3)
                # split the load across DMA queues for parallelism; finer
                # split for the first batch, whose loads pace the pipeline fill
                nsp = 4 if b == 0 else 2
                w_sp = e // nsp
                for qi in range(nsp):
                    nc.sync.dma_start(
                        x_sb[:sl, qi * w_sp:(qi + 1) * w_sp],
                        x[b, s0:s0 + sl, qi * w_sp:(qi + 1) * w_sp],
                    )
                # bf16 transposes run 1.0 cyc/row on PE (vs 1.5 fp32r);
                # convert x on ACT first. All 8 fit one full PSUM bank in
                # bf16 (half-bank tiles would fatally share banks between
                # PE writes and DVE reads).
                x16_sb = pool.tile([P, e], BF16, tag="x16", bufs=3)
                nc.scalar.copy(x16_sb[:sl, :], x_sb[:sl, :])
                tp = pp_tp.tile([P, n_e * P], BF16, tag="tp")
                for j in range(n_e):
                    nc.tensor.transpose(
                        tp[:, j * P:j * P + sl],
                        x16_sb[:sl, j * P:(j + 1) * P],
                        ident[:sl, :sl],
                    )
                tp3 = tp.rearrange("p (j c) -> p j c", c=P)
                nc.vector.tensor_copy(
                    xT8_all[:, :, s0:s0 + sl], tp3[:, :, :sl]
                )
                if s0 == 0:
                    nc.vector.tensor_copy(xT16_t0[:, :, :], tp3[:, :, :P])

        def stage_b(b, xT8_all, xT16_t0):
            # ---- stage B: q/k projections, fp8 DoubleRow ----
            # qT/kT [A, S] (a on partitions). Stationary = W pair tile
            # [P, 2, 128], reused across both S-chunks (one LDWEIGHTS per
            # 2 matmuls); moving = xT8 u-pair [P, 2, chunk].
            qkT = {}
            sch = _chunks(0, s, 512)
            for nm in ("q", "k"):
                tiles = []
                for m in range(n_a):
                    dest = pool.tile([P, s], BF16, tag=f"{nm}T", bufs=2 * n_a,
                                     name=f"{nm}T{b}_{m}")
                    tiles.append(dest)
                    mm = [pp_proj.tile([P, 512], F32, tag="proj",
                                       name=f"mm{ci}")
                          for ci in range(len(sch))]
                    for j in range(n_e // 2):
                        for ci, (c0, cl) in enumerate(sch):
                            nc.tensor.matmul(
                                mm[ci][:, :cl],
                                w8il[nm][:, j, m],
                                xT8_all[:, 2 * j:2 * j + 2, c0:c0 + cl],
                                start=(j == 0), stop=(j == n_e // 2 - 1),
                                perf_mode=DRS,
                            )
                    for ci, (c0, cl) in enumerate(sch):
                        nc.scalar.activation(
                            dest[:, c0:c0 + cl], mm[ci][:, :cl], AF.Identity,
                            bias=bias_sb[nm][:, m:m + 1],
                            scale=drain_scale[nm],
                        )
                qkT[nm] = tiles

            # v [S, A+2] natural layout, bf16; last two columns are ones (for
            # the softmax row-sums via the PV matmul). Projection runs in
            # fp8 DoubleRow (stationary = xT8 u-pair, moving = Wv8 pair);
            # rows >=128 average the fp8 v error over >=129 keys, and rows
            # 0..127 use a separate bf16 v (v16_t0) instead.
            v_tiles = []
            for (s0, sl) in s_tiles:
                vm = pp_proj.tile([P, 512], F32, tag="proj")
                for j in range(n_e // 2):
                    nc.tensor.matmul(
                        vm[:sl, :a],
                        xT8_all[:, 2 * j:2 * j + 2, s0:s0 + sl],
                        w8["v"][:, 2 * j:2 * j + 2, :],
                        start=(j == 0), stop=(j == n_e // 2 - 1),
                        perf_mode=DR,
                    )
                v_t = pool.tile([P, a + 2], BF16, tag="v", bufs=2 * n_s)
                nc.vector.tensor_scalar_mul(v_t[:sl, :a], vm[:sl, :a],
                                            1.0 / W8_SCALE)
                nc.gpsimd.memset(v_t[:sl, a:a + 2], 1.0)
                v_tiles.append(v_t)
            # bf16 v for q-tile 0 (output rows 0..127)
            vm0 = pp_proj.tile([P, 512], F32, tag="proj")
            for u in range(n_e):
                nc.tensor.matmul(
                    vm0[:P, :a], xT16_t0[:, u], w16v[u][:],
                    start=(u == 0), stop=(u == n_e - 1),
                )
            v16_t0 = pool.tile([P, a + 2], BF16, tag="v16t0", bufs=2)
            nc.vector.tensor_copy(v16_t0[:, :a], vm0[:, :a])
            nc.gpsimd.memset(v16_t0[:, a:a + 2], 1.0)
            return qkT, v_tiles, v16_t0

        def stage_cd(b, qkT, v_tiles, v16_t0):
            # ---- stages C+D interleaved per tile: scoresT/exp for k-tile
            # t, then PV/out for q-tile t (its expT deps are all ready) ----
            expT = []
            for t, (k0, kl) in enumerate(s_tiles):
                et = pool.tile([P, s - k0], BF16, tag=f"expT{t}", bufs=2,
                               name=f"et{b}_{t}")
                expT.append(et)
                for pi, (c0, cl) in enumerate(_chunks(k0, s - k0, 512)):
                    sc = pp_score.tile([P, 512], F32, tag="score")
                    for m in range(n_a):
                        nc.tensor.matmul(
                            sc[:kl, :cl],
                            qkT["k"][m][:, k0:k0 + kl],
                            qkT["q"][m][:, c0:c0 + cl],
                            start=(m == 0), stop=(m == n_a - 1),
                        )
                    if pi == 0:
                        # diagonal block: additive causal mask in PSUM
                        nc.vector.tensor_add(
                            sc[:kl, :kl], sc[:kl, :kl], amask[:kl, :kl]
                        )
                    nc.scalar.activation(
                        et[:kl, c0 - k0:c0 - k0 + cl], sc[:kl, :cl], AF.Exp,
                    )

                i, (q0, il) = t, s_tiles[t]
                op1 = pp_o1.tile([P, h], F32, tag="op1")
                op2 = pp_o2.tile([P, a - h + 2], F32, tag="op2")
                for t in range(i + 1):
                    k0t, klt = s_tiles[t]
                    lhs = expT[t][:klt, q0 - k0t:q0 - k0t + il]
                    vt = v16_t0 if i == 0 else v_tiles[t]
                    nc.tensor.matmul(
                        op1[:il, :], lhs, vt[:klt, 0:h],
                        start=(t == 0), stop=(t == i),
                    )
                    nc.tensor.matmul(
                        op2[:il, :], lhs, vt[:klt, h:a + 2],
                        start=(t == 0), stop=(t == i),
                    )
                rec = pool.tile([P, 1], F32, tag="rec", bufs=2)
                nc.vector.reciprocal(rec[:il, :], op2[:il, a - h:a - h + 1])
                o_sb = pool.tile([P, a], F32, tag="o_sb", bufs=3)
                # epilogue split per half so scale/bias-add/store pipeline
                nc.vector.tensor_scalar_mul(
                    o_sb[:il, 0:h], op1[:il, :], rec[:il, 0:1])
                nc.gpsimd.tensor_add(
                    o_sb[:il, 0:h], o_sb[:il, 0:h], bv_bc[:il, 0:h])
                nc.sync.dma_start(out[b, q0:q0 + il, 0:h], o_sb[:il, 0:h])
                nc.vector.tensor_scalar_mul(
                    o_sb[:il, h:a], op2[:il, 0:a - h], rec[:il, 0:1])
                nc.gpsimd.tensor_add(
                    o_sb[:il, h:a], o_sb[:il, h:a], bv_bc[:il, h:a])
                nc.sync.dma_start(out[b, q0:q0 + il, h:a], o_sb[:il, h:a])

        # Plain per-batch emission (A, B, C+D). Hoisting the next batch's
        # stage A earlier in the engine FIFOs was tried and regresses:
        # hoisted transposes head-of-line block scores/PV whenever their
        # x DMA isn't done yet.
        for b in range(b_pc):
            st = stage_a_alloc(b)
            stage_a(b, *st, s_tiles)
            proj = stage_b(b, *st)
            stage_cd(b, *proj)

    nc.compile()
    return nc


_BUILT = {}


def _get_nc(b_pc, s, e, a):
    key = (b_pc, s, e, a)
    if key not in _BUILT:
        _BUILT[key] = build(b_pc, s, e, a)
    return _BUILT[key]


def _interleave_w(W, e, a):
    """Host-side DoubleRowSwInterleave weight prep: [j, m, K, 256] blocks,
    per row [A127, B127, ..., A0, B0] (A = E-rows 2j*128.., B = next 128,
    columns reversed), pre-scaled by W8_SCALE."""
    n_j, n_a = e // (2 * P), a // P
    out = np.empty((n_j, n_a, P, 2 * P), np.float32)
    Ws = np.asarray(W, np.float32) * W8_SCALE
    for j in range(n_j):
        A_ = Ws[2 * j * P:(2 * j + 1) * P]
        B_ = Ws[(2 * j + 1) * P:(2 * j + 2) * P]
        for m in range(n_a):
            out[j, m, :, 0::2] = A_[:, m * P:(m + 1) * P][:, ::-1]
            out[j, m, :, 1::2] = B_[:, m * P:(m + 1) * P][:, ::-1]
    return np.ascontiguousarray(out)


def run_sharded(inputs, b_pc, s, e, a, **run_kwargs):
    """Run the SPMD kernel over N_CORES cores, sharding batch dim of x."""
    x = np.ascontiguousarray(inputs["x"], dtype=np.float32)
    b_total = x.shape[0]
    assert b_total == b_pc * N_CORES
    shared = {
        "Wq": _interleave_w(inputs["Wq"], e, a),
        "Wk": _interleave_w(inputs["Wk"], e, a),
        "Wv": np.ascontiguousarray(inputs["Wv"], dtype=np.float32),
        "bq": np.ascontiguousarray(inputs["bq"], dtype=np.float32),
        "bk": np.ascontiguousarray(inputs["bk"], dtype=np.float32),
        "bv": np.ascontiguousarray(inputs["bv"], dtype=np.float32),
    }
    in_maps = [
        {"x": x[c * b_pc:(c + 1) * b_pc], **shared} for c in range(N_CORES)
    ]
    nc = _get_nc(b_pc, s, e, a)
    res = run_bass_kernel_spmd(nc, in_maps, core_ids=list(range(N_CORES)),
                               **run_kwargs)
    full = np.concatenate([res.results[c]["out"] for c in range(N_CORES)], axis=0)
    return full, res


def kernel(**inputs) -> np.ndarray:
    out, _ = run_sharded(inputs, B // N_CORES, S, E, A)
    return out


# revision 39
# speedup vs baseline: 1.0714x; 1.0714x over previous
"""Masked self-attention Trainium2 Bass kernel.

Reference computation (per batch b):
    q = x @ Wq + bq ; k = x @ Wk + bk ; v = x @ Wv + bv      # [S, A]
    scores = (q @ k.T) / sqrt(S)  with causal mask            # [S, S]
    out = softmax(scores, axis=-1) @ v                        # [S, A]

Sharding: data-parallel over batch across 8 NeuronCores (B=32 -> 4 per core),
weights replicated. No collectives.

Per-core design (mixed precision; tolerance gate is 2e-2 — measured HW
max-rel error 1.46e-2, dominated by the fp8 q/k path on few-key early rows):
  Matmul dtypes: fp32 LDWEIGHTS can't use fast-weight-load, so a 128-col
  stationary reload costs ~224ns and serializes against the matmul stream
  (measured gap 272ns for N=500 fp32r MMs vs 208ns streaming). bf16 weights
  FWL-load in ~97ns and hide completely; fp8e4+DoubleRow streams 2 rows/cyc
  (0.5 cyc/row), halving projection stream time (DR LDWEIGHTS ~135ns does
  stay mostly serial — measured 211ns/MM vs 104ns streaming).
  Stage A: DMA x[b] [S,E] fp32; fp32r PE-transpose (identity) -> PSUM, 4
           transposes per bank; one strided 3D-AP DVE copy drains straight
           to xT8 [E,S+pad] fp8e4 (pad keeps the u-stride 16B-aligned for
           DoubleRow pair APs); s-cols 0..128 also copied to bf16 (xT16_t0)
           for the v fixup.
  Stage B: qT/kT [A,S] via fp8e4 DoubleRow matmuls: stationary = Wq/Wk pair
           tiles (pre-scaled x512 so W ~0.02 clears fp8's 2^-6 min normal),
           moving = xT8 u-pair 3D AP; bias + 1/sqrt(S) + 1/512 folded into
           the ACT PSUM->SBUF drain -> bf16.
           v [S,A] via fp8e4 DoubleRow (stationary = xT8 pair, moving = Wv8
           pair), DVE drain with 1/512 -> bf16; no bias (softmax rows sum to
           1, bv is added to the final output); 2 ones-columns appended via
           memset. Rows 0..127 of the output are ~v verbatim, so a separate
           bf16 v (v16_t0, from xT16_t0 @ Wv16) serves q-tile 0's PV.
  Stage C: scoresT[k,q] = kT16.T @ qT16 per k-tile, causal-trimmed chunks;
           additive -1e9 mask on the diagonal block in PSUM (DVE); exp on ACT
           (PSUM->SBUF bf16). No max-subtraction: |scores| <~ 3.
  Stage D: interleaved with C per tile: out_psum = sum_t expT[t].T @ v[t] in
           two column chunks; ones-columns give softmax row-sums; DVE
           reciprocal + tensor_scalar row scale; GPSIMD adds broadcast bv;
           DMA out per 256-column half.
"""

import numpy as np
from contextlib import ExitStack

import concourse.bass as bass
import concourse.mybir as mybir
import concourse.tile as tile
from concourse import bacc
from concourse.bass_utils import run_bass_kernel_spmd
from concourse.masks import make_identity

P = 128
F32 = mybir.dt.float32
F32R = mybir.dt.float32r
BF16 = mybir.dt.bfloat16
FP8 = mybir.dt.float8e4
DR = mybir.MatmulPerfMode.DoubleRow
DRS = mybir.MatmulPerfMode.DoubleRowSwInterleave
AF = mybir.ActivationFunctionType

N_CORES = 8
B, S, E, A = 32, 1000, 1024, 512
MASK_NEG = -1.0e9
W8_SCALE = 512.0  # Wq/Wk pre-scale before fp8 cast (values ~0.02 vs 2^-6 min)
S8 = 1008  # fp8 xT row stride: 16B-aligned for DoubleRow pair APs


def _chunks(start, total, maxc):
    """Split [start, start+total) into ceil(total/maxc) near-even chunks."""
    n = max(1, -(-total // maxc))
    bounds = [start + (i * total) // n for i in range(n)]
    bounds.append(start + total)
    return [(bounds[i], bounds[i + 1] - bounds[i]) for i in range(n)]


def build(b_pc, s, e, a, reps=1):
    assert e % P == 0 and a % P == 0
    n_s = -(-s // P)
    n_e = e // P
    n_a = a // P
    inv_den = float(s) ** -0.5
    s_tiles = [(t * P, min(P, s - t * P)) for t in range(n_s)]
    h = a // 2  # PV column split: [0,h) and [h, a+2)

    n_j = e // (2 * P)
    nc = bacc.Bacc("TRN2")
    x = nc.dram_tensor("x", [b_pc, s, e], F32R, kind="ExternalInput").ap()
    # Wq/Wk arrive pre-scaled (x W8_SCALE) and pre-interleaved on the host
    # for DoubleRowSwInterleave: [j, m, K=128, 256] with per-row layout
    # [A127, B127, A126, B126, ..., A0, B0] (A = E-rows 2j*128.., B = next
    # 128, columns reversed). Wv arrives in natural [E, A] layout.
    w_dram = {
        "q": nc.dram_tensor("Wq", [n_j, a // P, P, 2 * P], F32,
                            kind="ExternalInput").ap(),
        "k": nc.dram_tensor("Wk", [n_j, a // P, P, 2 * P], F32,
                            kind="ExternalInput").ap(),
        "v": nc.dram_tensor("Wv", [e, a], F32, kind="ExternalInput").ap(),
    }
    b_dram = {
        "q": nc.dram_tensor("bq", [a], F32, kind="ExternalInput").ap(),
        "k": nc.dram_tensor("bk", [a], F32, kind="ExternalInput").ap(),
        "v": nc.dram_tensor("bv", [a], F32, kind="ExternalInput").ap(),
    }
    out = nc.dram_tensor("out", [b_pc, s, a], F32, kind="ExternalOutput").ap()

    with tile.TileContext(nc) as tc, ExitStack() as ctx:
        pool = ctx.enter_context(tc.tile_pool(name="sb", bufs=1))
        pp_tp = ctx.enter_context(tc.tile_pool(name="pp_tp", bufs=2, space="PSUM"))
        pp_proj = ctx.enter_context(tc.tile_pool(name="pp_proj", bufs=2, space="PSUM"))
        pp_score = ctx.enter_context(tc.tile_pool(name="pp_sc", bufs=2, space="PSUM"))
        pp_o1 = ctx.enter_context(tc.tile_pool(name="pp_o1", bufs=1, space="PSUM"))
        pp_o2 = ctx.enter_context(tc.tile_pool(name="pp_o2", bufs=1, space="PSUM"))

        # ---------------- constants ----------------
        ident_st = pool.tile([P, P], F32)
        make_identity(nc, ident_st)
        ident = pool.tile([P, P], F32R)
        nc.scalar.copy(ident[:], ident_st[:])

        # additive causal mask for the diagonal block:
        # keep 0 where col q >= row k (i.e. (y - x) >= 0), else fill -1e9
        amask = pool.tile([P, P], F32)
        nc.gpsimd.memset(amask, 0.0)
        nc.gpsimd.affine_select(
            out=amask, in_=amask,
            compare_op=mybir.AluOpType.is_ge,
            fill=MASK_NEG, base=0,
            pattern=[[1, P]], channel_multiplier=-1,
        )

        # ---------------- weights / biases ----------------
        # q/k/v weights: fp8e4 pair layout [P, u, a] (u-stride a bytes, 16B
        # aligned), pre-scaled by W8_SCALE. v also kept in bf16 for the
        # q-tile-0 fixup (early output rows are ~v verbatim, so the fp8 v
        # error would land on them directly).
        w8 = {}
        for nm in ("v",):
            w8_all = pool.tile([P, n_e, a], FP8, tag=f"w8_{nm}", bufs=1)
            for u in range(n_e):
                w_stage = pool.tile([P, a], F32, tag="w_stage", bufs=2)
                nc.gpsimd.dma_start(w_stage[:], w_dram[nm][u * P:(u + 1) * P, :])
                nc.scalar.mul(w8_all[:, u], w_stage[:], W8_SCALE)
            w8[nm] = w8_all
        # q/k: interleaved fp8 pair blocks for DoubleRowSwInterleave (host
        # pre-scaled by W8_SCALE; device just casts)
        w8il = {}
        for nm in ("q", "k"):
            il_all = pool.tile([P, n_j, n_a, 2 * P], FP8, tag=f"w8il_{nm}",
                               bufs=1)
            for j in range(n_j):
                for m in range(n_a):
                    w_stage = pool.tile([P, 2 * P], F32, tag="wil_stage",
                                        bufs=2)
                    nc.gpsimd.dma_start(w_stage[:], w_dram[nm][j, m])
                    nc.scalar.copy(il_all[:, j, m], w_stage[:])
            w8il[nm] = il_all
        w16v = []
        for u in range(n_e):
            w_stage = pool.tile([P, a], F32, tag="w_stage", bufs=2)
            nc.gpsimd.dma_start(w_stage[:], w_dram["v"][u * P:(u + 1) * P, :])
            w_r = pool.tile([P, a], BF16, tag="w16_v", bufs=n_e)
            nc.vector.tensor_copy(w_r[:], w_stage[:])
            w16v.append(w_r)

        bias_sb = {}
        for nm in ("q", "k"):
            b_st = pool.tile([P, n_a], F32, tag=f"b_{nm}", bufs=1)
            nc.gpsimd.dma_start(
                b_st[:], b_dram[nm].rearrange("(m p) -> p m", p=P)
            )
            bias_sb[nm] = b_st
        # pre-scale bq by 1/sqrt(S) (scores scaling folded into q)
        bqs = pool.tile([P, n_a], F32)
        nc.scalar.mul(bqs[:], bias_sb["q"][:], inv_den)
        bias_sb["q"] = bqs
        # PSUM-drain scales (undo the fp8 weight pre-scale)
        drain_scale = {"q": inv_den / W8_SCALE, "k": 1.0 / W8_SCALE}

        bv_stage = pool.tile([1, a], F32)
        nc.gpsimd.dma_start(bv_stage[:], b_dram["v"][:])
        bv_bc = pool.tile([P, a], F32)
        nc.gpsimd.partition_broadcast(bv_bc[:], bv_stage[:])

        # ---------------- per-batch pipeline ----------------
        # reps>1 wraps the whole pipeline in an on-device loop re-running the
        # same work — used only to measure device exec time (amortizes the
        # per-dispatch RPC overhead, which otherwise hides the kernel).
        rep_ctx = (tc.For_i(0, reps, 1, hint_engines=tuple(nc.engines),
                            staggered_reset=True)
                   if reps > 1 else None)
        if rep_ctx is not None:
            ctx.enter_context(rep_ctx)

        def stage_a_alloc(b):
            # stage A outputs: xT8 [E,S] fp8; a small bf16 slice (s-cols
            # 0..P) feeds the bf16 v fixup for q-tile 0
            xT16_t0 = pool.tile([P, n_e, P], BF16, tag="xT16t0", bufs=2,
                                name=f"xT16t0_{b}")
            xT8_all = pool.tile([P, n_e, S8], FP8, tag="xT8", bufs=2,
                                name=f"xT8_{b}")
            return xT8_all, xT16_t0

        def stage_a(b, xT8_all, xT16_t0, tiles):
            # ---- stage A: load x, transpose, drain to fp8 SBUF ----
            for (s0, sl) in tiles:
                x_sb = pool.tile([P, e], F32R, tag="x", bufs=3)
                # split the load across DMA queues for parallelism; finer
                # split for the first batch, whose loads pace the pipeline fill
                nsp = 4 if b == 0 else 2
                w_sp = e // nsp
                for qi in range(nsp):
                    nc.sync.dma_start(
                        x_sb[:sl, qi * w_sp:(qi + 1) * w_sp],
                        x[b, s0:s0 + sl, qi * w_sp:(qi + 1) * w_sp],
                    )
                # fp32r transposes (1.5 cyc/row): bf16 transposes were tried
                # — PE busy drops 7us but an ACT x->bf16 conversion enters
                # the fill path, growing the rep-boundary bubble by ~3.5us
                # and costing 0.11e-2 error (double rounding). Net wash,
                # worse margin. 4 transposes share one PSUM bank; one
                # strided 3D-AP DVE copy drains straight to fp8.
                for u0 in range(0, n_e, 4):
                    tp = pp_tp.tile([P, 4 * P], F32R, tag="tp")
                    for j in range(4):
                        nc.tensor.transpose(
                            tp[:, j * P:j * P + sl],
                            x_sb[:sl, (u0 + j) * P:(u0 + j + 1) * P],
                            ident[:sl, :sl],
                        )
                    tp3 = tp.rearrange("p (j c) -> p j c", c=P)
                    nc.vector.tensor_copy(
                        xT8_all[:, u0:u0 + 4, s0:s0 + sl], tp3[:, :, :sl]
                    )
                    if s0 == 0:
                        nc.vector.tensor_copy(
                            xT16_t0[:, u0:u0 + 4, :], tp3[:, :, :P]
                        )

        def stage_b(b, xT8_all, xT16_t0):
            # ---- stage B: q/k projections, fp8 DoubleRow ----
            # qT/kT [A, S] (a on partitions). Stationary = W pair tile
            # [P, 2, 128], reused across both S-chunks (one LDWEIGHTS per
            # 2 matmuls); moving = xT8 u-pair [P, 2, chunk].
            qkT = {}
            sch = _chunks(0, s, 512)
            for nm in ("q", "k"):
                tiles = []
                for m in range(n_a):
                    dest = pool.tile([P, s], BF16, tag=f"{nm}T", bufs=2 * n_a,
                                     name=f"{nm}T{b}_{m}")
                    tiles.append(dest)
                    mm = [pp_proj.tile([P, 512], F32, tag="proj",
                                       name=f"mm{ci}")
                          for ci in range(len(sch))]
                    for j in range(n_e // 2):
                        for ci, (c0, cl) in enumerate(sch):
                            nc.tensor.matmul(
                                mm[ci][:, :cl],
                                w8il[nm][:, j, m],
                                xT8_all[:, 2 * j:2 * j + 2, c0:c0 + cl],
                                start=(j == 0), stop=(j == n_e // 2 - 1),
                                perf_mode=DRS,
                            )
                    for ci, (c0, cl) in enumerate(sch):
                        nc.scalar.activation(
                            dest[:, c0:c0 + cl], mm[ci][:, :cl], AF.Identity,
                            bias=bias_sb[nm][:, m:m + 1],
                            scale=drain_scale[nm],
                        )
                qkT[nm] = tiles

            # v [S, A+2] natural layout, bf16; last two columns are ones (for
            # the softmax row-sums via the PV matmul). Projection runs in
            # fp8 DoubleRow (stationary = xT8 u-pair, moving = Wv8 pair);
            # rows >=128 average the fp8 v error over >=129 keys, and rows
            # 0..127 use a separate bf16 v (v16_t0) instead.
            v_tiles = []
            for (s0, sl) in s_tiles:
                vm = pp_proj.tile([P, 512], F32, tag="proj")
                for j in range(n_e // 2):
                    nc.tensor.matmul(
                        vm[:sl, :a],
                        xT8_all[:, 2 * j:2 * j + 2, s0:s0 + sl],
                        w8["v"][:, 2 * j:2 * j + 2, :],
                        start=(j == 0), stop=(j == n_e // 2 - 1),
                        perf_mode=DR,
                    )
                v_t = pool.tile([P, a + 2], BF16, tag="v", bufs=2 * n_s)
                nc.vector.tensor_scalar_mul(v_t[:sl, :a], vm[:sl, :a],
                                            1.0 / W8_SCALE)
                nc.gpsimd.memset(v_t[:sl, a:a + 2], 1.0)
                v_tiles.append(v_t)
            # bf16 v for q-tile 0 (output rows 0..127)
            vm0 = pp_proj.tile([P, 512], F32, tag="proj")
            for u in range(n_e):
                nc.tensor.matmul(
                    vm0[:P, :a], xT16_t0[:, u], w16v[u][:],
                    start=(u == 0), stop=(u == n_e - 1),
                )
            v16_t0 = pool.tile([P, a + 2], BF16, tag="v16t0", bufs=2)
            nc.vector.tensor_copy(v16_t0[:, :a], vm0[:, :a])
            nc.gpsimd.memset(v16_t0[:, a:a + 2], 1.0)
            return qkT, v_tiles, v16_t0

        def stage_cd(b, qkT, v_tiles, v16_t0):
            # ---- stages C+D interleaved per tile: scoresT/exp for k-tile
            # t, then PV/out for q-tile t (its expT deps are all ready) ----
            expT = []
            for t, (k0, kl) in enumerate(s_tiles):
                et = pool.tile([P, s - k0], BF16, tag=f"expT{t}", bufs=2,
                               name=f"et{b}_{t}")
                expT.append(et)
                for pi, (c0, cl) in enumerate(_chunks(k0, s - k0, 512)):
                    sc = pp_score.tile([P, 512], F32, tag="score")
                    for m in range(n_a):
                        nc.tensor.matmul(
                            sc[:kl, :cl],
                            qkT["k"][m][:, k0:k0 + kl],
                            qkT["q"][m][:, c0:c0 + cl],
                            start=(m == 0), stop=(m == n_a - 1),
                        )
                    if pi == 0:
                        # diagonal block: additive causal mask in PSUM
                        nc.vector.tensor_add(
                            sc[:kl, :kl], sc[:kl, :kl], amask[:kl, :kl]
                        )
                    nc.scalar.activation(
                        et[:kl, c0 - k0:c0 - k0 + cl], sc[:kl, :cl], AF.Exp,
                    )

                i, (q0, il) = t, s_tiles[t]
                op1 = pp_o1.tile([P, h], F32, tag="op1")
                op2 = pp_o2.tile([P, a - h + 2], F32, tag="op2")
                for t in range(i + 1):
                    k0t, klt = s_tiles[t]
                    lhs = expT[t][:klt, q0 - k0t:q0 - k0t + il]
                    vt = v16_t0 if i == 0 else v_tiles[t]
                    nc.tensor.matmul(
                        op1[:il, :], lhs, vt[:klt, 0:h],
                        start=(t == 0), stop=(t == i),
                    )
                    nc.tensor.matmul(
                        op2[:il, :], lhs, vt[:klt, h:a + 2],
                        start=(t == 0), stop=(t == i),
                    )
                rec = pool.tile([P, 1], F32, tag="rec", bufs=2)
                nc.vector.reciprocal(rec[:il, :], op2[:il, a - h:a - h + 1])
                o_sb = pool.tile([P, a], F32, tag="o_sb", bufs=3)
                # epilogue split per half so scale/bias-add/store pipeline
                nc.vector.tensor_scalar_mul(
                    o_sb[:il, 0:h], op1[:il, :], rec[:il, 0:1])
                nc.gpsimd.tensor_add(
                    o_sb[:il, 0:h], o_sb[:il, 0:h], bv_bc[:il, 0:h])
                nc.sync.dma_start(out[b, q0:q0 + il, 0:h], o_sb[:il, 0:h])
                nc.vector.tensor_scalar_mul(
                    o_sb[:il, h:a], op2[:il, 0:a - h], rec[:il, 0:1])
                nc.gpsimd.tensor_add(
                    o_sb[:il, h:a], o_sb[:il, h:a], bv_bc[:il, h:a])
                nc.sync.dma_start(out[b, q0:q0 + il, h:a], o_sb[:il, h:a])

        # Plain per-batch emission (A, B, C+D). Hoisting the next batch's
        # stage A earlier in the engine FIFOs was tried and regresses:
        # hoisted transposes head-of-line block scores/PV whenever their
        # x DMA isn't done yet.
        for b in range(b_pc):
            st = stage_a_alloc(b)
            stage_a(b, *st, s_tiles)
            proj = stage_b(b, *st)
            stage_cd(b, *proj)

    nc.compile()
    return nc


_BUILT = {}


def _get_nc(b_pc, s, e, a):
    key = (b_pc, s, e, a)
    if key not in _BUILT:
        _BUILT[key] = build(b_pc, s, e, a)
    return _BUILT[key]


def _interleave_w(W, e, a):
    """Host-side DoubleRowSwInterleave weight prep: [j, m, K, 256] blocks,
    per row [A127, B127, ..., A0, B0] (A = E-rows 2j*128.., B = next 128,
    columns reversed), pre-scaled by W8_SCALE."""
    n_j, n_a = e // (2 * P), a // P
    out = np.empty((n_j, n_a, P, 2 * P), np.float32)
    Ws = np.asarray(W, np.float32) * W8_SCALE
    for j in range(n_j):
        A_ = Ws[2 * j * P:(2 * j + 1) * P]
        B_ = Ws[(2 * j + 1) * P:(2 * j + 2) * P]
        for m in range(n_a):
            out[j, m, :, 0::2] = A_[:, m * P:(m + 1) * P][:, ::-1]
            out[j, m, :, 1::2] = B_[:, m * P:(m + 1) * P][:, ::-1]
    return np.ascontiguousarray(out)


def run_sharded(inputs, b_pc, s, e, a, **run_kwargs):
    """Run the SPMD kernel over N_CORES cores, sharding batch dim of x."""
    x = np.ascontiguousarray(inputs["x"], dtype=np.float32)
    b_total = x.shape[0]
    assert b_total == b_pc * N_CORES
    shared = {
        "Wq": _interleave_w(inputs["Wq"], e, a),
        "Wk": _interleave_w(inputs["Wk"], e, a),
        "Wv": np.ascontiguousarray(inputs["Wv"], dtype=np.float32),
        "bq": np.ascontiguousarray(inputs["bq"], dtype=np.float32),
        "bk": np.ascontiguousarray(inputs["bk"], dtype=np.float32),
        "bv": np.ascontiguousarray(inputs["bv"], dtype=np.float32),
    }
    in_maps = [
        {"x": x[c * b_pc:(c + 1) * b_pc], **shared} for c in range(N_CORES)
    ]
    nc = _get_nc(b_pc, s, e, a)
    res = run_bass_kernel_spmd(nc, in_maps, core_ids=list(range(N_CORES)),
                               **run_kwargs)
    full = np.concatenate([res.results[c]["out"] for c in range(N_CORES)], axis=0)
    return full, res


def kernel(**inputs) -> np.ndarray:
    out, _ = run_sharded(inputs, B // N_CORES, S, E, A)
    return out


# revision 41
# speedup vs baseline: 3.1105x; 2.9032x over previous
"""Masked self-attention Trainium2 Bass kernel.

Reference computation (per batch b):
    q = x @ Wq + bq ; k = x @ Wk + bk ; v = x @ Wv + bv      # [S, A]
    scores = (q @ k.T) / sqrt(S)  with causal mask            # [S, S]
    out = softmax(scores, axis=-1) @ v                        # [S, A]

Sharding: data-parallel over batch across 8 NeuronCores (B=32 -> 4 per core),
weights replicated. No collectives.

Per-core design (mixed precision; tolerance gate is 2e-2 — measured HW
max-rel error 1.46e-2, dominated by the fp8 q/k path on few-key early rows):
  Matmul dtypes: fp32 LDWEIGHTS can't use fast-weight-load, so a 128-col
  stationary reload costs ~224ns and serializes against the matmul stream
  (measured gap 272ns for N=500 fp32r MMs vs 208ns streaming). bf16 weights
  FWL-load in ~97ns and hide completely; fp8e4+DoubleRow streams 2 rows/cyc
  (0.5 cyc/row), halving projection stream time (DR LDWEIGHTS ~135ns does
  stay mostly serial — measured 211ns/MM vs 104ns streaming).
  Stage A: DMA x[b] [S,E] fp32; fp32r PE-transpose (identity) -> PSUM, 4
           transposes per bank; one strided 3D-AP DVE copy drains straight
           to xT8 [E,S+pad] fp8e4 (pad keeps the u-stride 16B-aligned for
           DoubleRow pair APs); s-cols 0..128 also copied to bf16 (xT16_t0)
           for the v fixup.
  Stage B: qT/kT [A,S] via fp8e4 DoubleRow matmuls: stationary = Wq/Wk pair
           tiles (pre-scaled x512 so W ~0.02 clears fp8's 2^-6 min normal),
           moving = xT8 u-pair 3D AP; bias + 1/sqrt(S) + 1/512 folded into
           the ACT PSUM->SBUF drain -> bf16.
           v [S,A] via fp8e4 DoubleRow (stationary = xT8 pair, moving = Wv8
           pair), DVE drain with 1/512 -> bf16; no bias (softmax rows sum to
           1, bv is added to the final output); 2 ones-columns appended via
           memset. Rows 0..127 of the output are ~v verbatim, so a separate
           bf16 v (v16_t0, from xT16_t0 @ Wv16) serves q-tile 0's PV.
  Stage C: scoresT[k,q] = kT16.T @ qT16 per k-tile, causal-trimmed chunks;
           additive -1e9 mask on the diagonal block in PSUM (DVE); exp on ACT
           (PSUM->SBUF bf16). No max-subtraction: |scores| <~ 3.
  Stage D: interleaved with C per tile: out_psum = sum_t expT[t].T @ v[t] in
           two column chunks; ones-columns give softmax row-sums; DVE
           reciprocal + tensor_scalar row scale; GPSIMD adds broadcast bv;
           DMA out per 256-column half.
"""

import numpy as np
from contextlib import ExitStack

import concourse.bass as bass
import concourse.mybir as mybir
import concourse.tile as tile
from concourse import bacc
from concourse.bass_utils import run_bass_kernel_spmd
from concourse.masks import make_identity

P = 128
F32 = mybir.dt.float32
F32R = mybir.dt.float32r
BF16 = mybir.dt.bfloat16
FP8 = mybir.dt.float8e4
DR = mybir.MatmulPerfMode.DoubleRow
DRS = mybir.MatmulPerfMode.DoubleRowSwInterleave
AF = mybir.ActivationFunctionType

N_CORES = 8
B, S, E, A = 32, 1000, 1024, 512
MASK_NEG = -1.0e9
W8_SCALE = 512.0  # Wq/Wk pre-scale before fp8 cast (values ~0.02 vs 2^-6 min)
S8 = 1008  # fp8 xT row stride: 16B-aligned for DoubleRow pair APs


def _chunks(start, total, maxc):
    """Split [start, start+total) into ceil(total/maxc) near-even chunks."""
    n = max(1, -(-total // maxc))
    bounds = [start + (i * total) // n for i in range(n)]
    bounds.append(start + total)
    return [(bounds[i], bounds[i + 1] - bounds[i]) for i in range(n)]


def build(b_pc, s, e, a, reps=1):
    assert e % P == 0 and a % P == 0
    n_s = -(-s // P)
    n_e = e // P
    n_a = a // P
    inv_den = float(s) ** -0.5
    s_tiles = [(t * P, min(P, s - t * P)) for t in range(n_s)]
    h = a // 2  # PV column split: [0,h) and [h, a+2)

    n_j = e // (2 * P)
    nc = bacc.Bacc("TRN2")
    x = nc.dram_tensor("x", [b_pc, s, e], F32R, kind="ExternalInput").ap()
    # Wq/Wk arrive pre-scaled (x W8_SCALE) and pre-interleaved on the host
    # for DoubleRowSwInterleave: [j, m, K=128, 256] with per-row layout
    # [A127, B127, A126, B126, ..., A0, B0] (A = E-rows 2j*128.., B = next
    # 128, columns reversed). Wv arrives in natural [E, A] layout.
    w_dram = {
        "q": nc.dram_tensor("Wq", [n_j, a // P, P, 2 * P], F32,
                            kind="ExternalInput").ap(),
        "k": nc.dram_tensor("Wk", [n_j, a // P, P, 2 * P], F32,
                            kind="ExternalInput").ap(),
        "v": nc.dram_tensor("Wv", [e, a], F32, kind="ExternalInput").ap(),
    }
    b_dram = {
        "q": nc.dram_tensor("bq", [a], F32, kind="ExternalInput").ap(),
        "k": nc.dram_tensor("bk", [a], F32, kind="ExternalInput").ap(),
        "v": nc.dram_tensor("bv", [a], F32, kind="ExternalInput").ap(),
    }
    out = nc.dram_tensor("out", [b_pc, s, a], F32, kind="ExternalOutput").ap()

    with tile.TileContext(nc) as tc, ExitStack() as ctx:
        pool = ctx.enter_context(tc.tile_pool(name="sb", bufs=1))
        pp_tp = ctx.enter_context(tc.tile_pool(name="pp_tp", bufs=2, space="PSUM"))
        pp_proj = ctx.enter_context(tc.tile_pool(name="pp_proj", bufs=2, space="PSUM"))
        pp_score = ctx.enter_context(tc.tile_pool(name="pp_sc", bufs=2, space="PSUM"))
        pp_o1 = ctx.enter_context(tc.tile_pool(name="pp_o1", bufs=1, space="PSUM"))
        pp_o2 = ctx.enter_context(tc.tile_pool(name="pp_o2", bufs=1, space="PSUM"))

        # ---------------- constants ----------------
        ident_st = pool.tile([P, P], F32)
        make_identity(nc, ident_st)
        ident = pool.tile([P, P], F32R)
        nc.scalar.copy(ident[:], ident_st[:])

        # additive causal mask for the diagonal block:
        # keep 0 where col q >= row k (i.e. (y - x) >= 0), else fill -1e9
        amask = pool.tile([P, P], F32)
        nc.gpsimd.memset(amask, 0.0)
        nc.gpsimd.affine_select(
            out=amask, in_=amask,
            compare_op=mybir.AluOpType.is_ge,
            fill=MASK_NEG, base=0,
            pattern=[[1, P]], channel_multiplier=-1,
        )

        # ---------------- weights / biases ----------------
        # q/k/v weights: fp8e4 pair layout [P, u, a] (u-stride a bytes, 16B
        # aligned), pre-scaled by W8_SCALE. v also kept in bf16 for the
        # q-tile-0 fixup (early output rows are ~v verbatim, so the fp8 v
        # error would land on them directly).
        w8 = {}
        for nm in ("v",):
            w8_all = pool.tile([P, n_e, a], FP8, tag=f"w8_{nm}", bufs=1)
            for u in range(n_e):
                w_stage = pool.tile([P, a], F32, tag="w_stage", bufs=2)
                nc.gpsimd.dma_start(w_stage[:], w_dram[nm][u * P:(u + 1) * P, :])
                nc.scalar.mul(w8_all[:, u], w_stage[:], W8_SCALE)
            w8[nm] = w8_all
        # q/k: interleaved fp8 pair blocks for DoubleRowSwInterleave (host
        # pre-scaled by W8_SCALE; device just casts)
        w8il = {}
        for nm in ("q", "k"):
            il_all = pool.tile([P, n_j, n_a, 2 * P], FP8, tag=f"w8il_{nm}",
                               bufs=1)
            for j in range(n_j):
                for m in range(n_a):
                    w_stage = pool.tile([P, 2 * P], F32, tag="wil_stage",
                                        bufs=2)
                    nc.gpsimd.dma_start(w_stage[:], w_dram[nm][j, m])
                    nc.scalar.copy(il_all[:, j, m], w_stage[:])
            w8il[nm] = il_all
        w16v = []
        for u in range(n_e):
            w_stage = pool.tile([P, a], F32, tag="w_stage", bufs=2)
            nc.gpsimd.dma_start(w_stage[:], w_dram["v"][u * P:(u + 1) * P, :])
            w_r = pool.tile([P, a], BF16, tag="w16_v", bufs=n_e)
            nc.vector.tensor_copy(w_r[:], w_stage[:])
            w16v.append(w_r)

        bias_sb = {}
        for nm in ("q", "k"):
            b_st = pool.tile([P, n_a], F32, tag=f"b_{nm}", bufs=1)
            nc.gpsimd.dma_start(
                b_st[:], b_dram[nm].rearrange("(m p) -> p m", p=P)
            )
            bias_sb[nm] = b_st
        # pre-scale bq by 1/sqrt(S) (scores scaling folded into q)
        bqs = pool.tile([P, n_a], F32)
        nc.scalar.mul(bqs[:], bias_sb["q"][:], inv_den)
        bias_sb["q"] = bqs
        # PSUM-drain scales (undo the fp8 weight pre-scale)
        drain_scale = {"q": inv_den / W8_SCALE, "k": 1.0 / W8_SCALE}

        bv_stage = pool.tile([1, a], F32)
        nc.gpsimd.dma_start(bv_stage[:], b_dram["v"][:])
        bv_bc = pool.tile([P, a], F32)
        nc.gpsimd.partition_broadcast(bv_bc[:], bv_stage[:])

        # ---------------- per-batch pipeline ----------------
        # reps>1 wraps the whole pipeline in an on-device loop re-running the
        # same work — used only to measure device exec time (amortizes the
        # per-dispatch RPC overhead, which otherwise hides the kernel).
        rep_ctx = (tc.For_i(0, reps, 1, hint_engines=tuple(nc.engines),
                            staggered_reset=True)
                   if reps > 1 else None)
        if rep_ctx is not None:
            ctx.enter_context(rep_ctx)

        def stage_a_alloc(b):
            # stage A outputs: xT8 [E,S] fp8; a small bf16 slice (s-cols
            # 0..P) feeds the bf16 v fixup for q-tile 0
            xT16_t0 = pool.tile([P, n_e, P], BF16, tag="xT16t0", bufs=2,
                                name=f"xT16t0_{b}")
            xT8_all = pool.tile([P, n_e, S8], FP8, tag="xT8", bufs=2,
                                name=f"xT8_{b}")
            return xT8_all, xT16_t0

        def stage_a(b, xT8_all, xT16_t0, tiles):
            # ---- stage A: load x, transpose, drain to fp8 SBUF ----
            for (s0, sl) in tiles:
                x_sb = pool.tile([P, e], F32R, tag="x", bufs=3)
                # split the load across DMA queues for parallelism; finer
                # split for the first batch, whose loads pace the pipeline fill
                nsp = 4 if b == 0 else 2
                w_sp = e // nsp
                for qi in range(nsp):
                    nc.sync.dma_start(
                        x_sb[:sl, qi * w_sp:(qi + 1) * w_sp],
                        x[b, s0:s0 + sl, qi * w_sp:(qi + 1) * w_sp],
                    )
                # fp32r transposes (1.5 cyc/row): bf16 transposes were tried
                # — PE busy drops 7us but an ACT x->bf16 conversion enters
                # the fill path, growing the rep-boundary bubble by ~3.5us
                # and costing 0.11e-2 error (double rounding). Net wash,
                # worse margin. 4 transposes share one PSUM bank; one
                # strided 3D-AP DVE copy drains straight to fp8.
                for u0 in range(0, n_e, 4):
                    tp = pp_tp.tile([P, 4 * P], F32R, tag="tp")
                    for j in range(4):
                        nc.tensor.transpose(
                            tp[:, j * P:j * P + sl],
                            x_sb[:sl, (u0 + j) * P:(u0 + j + 1) * P],
                            ident[:sl, :sl],
                        )
                    tp3 = tp.rearrange("p (j c) -> p j c", c=P)
                    nc.vector.tensor_copy(
                        xT8_all[:, u0:u0 + 4, s0:s0 + sl], tp3[:, :, :sl]
                    )
                    if s0 == 0:
                        nc.vector.tensor_copy(
                            xT16_t0[:, u0:u0 + 4, :], tp3[:, :, :P]
                        )

        def stage_b(b, xT8_all, xT16_t0):
            # ---- stage B: q/k projections, fp8 DoubleRow ----
            # qT/kT [A, S] (a on partitions). Stationary = W pair tile
            # [P, 2, 128], reused across both S-chunks (one LDWEIGHTS per
            # 2 matmuls); moving = xT8 u-pair [P, 2, chunk].
            # qT/kT stored fp8e4 in m-major pair layout [P, n_a, S8] so the
            # scores matmuls can run DoubleRow over m-pairs (contraction A);
            # single rounding from PSUM f32.
            qkT = {}
            sch = _chunks(0, s, 512)
            for nm in ("q", "k"):
                dest = pool.tile([P, n_a, S8], FP8, tag=f"{nm}T8", bufs=2,
                                 name=f"{nm}T8_{b}")
                for m in range(n_a):
                    mm = [pp_proj.tile([P, 512], F32, tag="proj",
                                       name=f"mm{ci}")
                          for ci in range(len(sch))]
                    for j in range(n_e // 2):
                        for ci, (c0, cl) in enumerate(sch):
                            nc.tensor.matmul(
                                mm[ci][:, :cl],
                                w8il[nm][:, j, m],
                                xT8_all[:, 2 * j:2 * j + 2, c0:c0 + cl],
                                start=(j == 0), stop=(j == n_e // 2 - 1),
                                perf_mode=DRS,
                            )
                    for ci, (c0, cl) in enumerate(sch):
                        nc.scalar.activation(
                            dest[:, m, c0:c0 + cl], mm[ci][:, :cl],
                            AF.Identity,
                            bias=bias_sb[nm][:, m:m + 1],
                            scale=drain_scale[nm],
                        )
                qkT[nm] = dest

            # v [S, A+2] natural layout, bf16; last two columns are ones (for
            # the softmax row-sums via the PV matmul). Projection runs in
            # fp8 DoubleRow (stationary = xT8 u-pair, moving = Wv8 pair);
            # rows >=128 average the fp8 v error over >=129 keys, and rows
            # 0..127 use a separate bf16 v (v16_t0) instead.
            v_tiles = []
            for (s0, sl) in s_tiles:
                vm = pp_proj.tile([P, 512], F32, tag="proj")
                for j in range(n_e // 2):
                    nc.tensor.matmul(
                        vm[:sl, :a],
                        xT8_all[:, 2 * j:2 * j + 2, s0:s0 + sl],
                        w8["v"][:, 2 * j:2 * j + 2, :],
                        start=(j == 0), stop=(j == n_e // 2 - 1),
                        perf_mode=DR,
                    )
                v_t = pool.tile([P, a + 2], BF16, tag="v", bufs=2 * n_s)
                nc.vector.tensor_scalar_mul(v_t[:sl, :a], vm[:sl, :a],
                                            1.0 / W8_SCALE)
                nc.gpsimd.memset(v_t[:sl, a:a + 2], 1.0)
                v_tiles.append(v_t)
            # bf16 v for q-tile 0 (output rows 0..127)
            vm0 = pp_proj.tile([P, 512], F32, tag="proj")
            for u in range(n_e):
                nc.tensor.matmul(
                    vm0[:P, :a], xT16_t0[:, u], w16v[u][:],
                    start=(u == 0), stop=(u == n_e - 1),
                )
            v16_t0 = pool.tile([P, a + 2], BF16, tag="v16t0", bufs=2)
            nc.vector.tensor_copy(v16_t0[:, :a], vm0[:, :a])
            nc.gpsimd.memset(v16_t0[:, a:a + 2], 1.0)
            return qkT, v_tiles, v16_t0

        def stage_cd(b, qkT, v_tiles, v16_t0):
            # ---- stages C+D interleaved per tile: scoresT/exp for k-tile
            # t, then PV/out for q-tile t (its expT deps are all ready) ----
            expT = []
            for t, (k0, kl) in enumerate(s_tiles):
                et = pool.tile([P, s - k0], BF16, tag=f"expT{t}", bufs=2,
                               name=f"et{b}_{t}")
                expT.append(et)
                for pi, (c0, cl) in enumerate(_chunks(k0, s - k0, 512)):
                    sc = pp_score.tile([P, 512], F32, tag="score")
                    if cl >= 256:
                        # DoubleRow over m-pairs: 2 MMs instead of 4.
                        # Below cl=256 the DR LDWEIGHTS floor (~211ns/MM)
                        # loses to plain fp8, whose FWL-loaded weights hide.
                        for i2 in range(n_a // 2):
                            nc.tensor.matmul(
                                sc[:kl, :cl],
                                qkT["k"][:, 2 * i2:2 * i2 + 2, k0:k0 + kl],
                                qkT["q"][:, 2 * i2:2 * i2 + 2, c0:c0 + cl],
                                start=(i2 == 0), stop=(i2 == n_a // 2 - 1),
                                perf_mode=DR,
                            )
                    else:
                        for m in range(n_a):
                            nc.tensor.matmul(
                                sc[:kl, :cl],
                                qkT["k"][:, m, k0:k0 + kl],
                                qkT["q"][:, m, c0:c0 + cl],
                                start=(m == 0), stop=(m == n_a - 1),
                            )
                    if pi == 0:
                        # diagonal block: additive causal mask in PSUM
                        nc.vector.tensor_add(
                            sc[:kl, :kl], sc[:kl, :kl], amask[:kl, :kl]
                        )
                    nc.scalar.activation(
                        et[:kl, c0 - k0:c0 - k0 + cl], sc[:kl, :cl], AF.Exp,
                    )

                i, (q0, il) = t, s_tiles[t]
                op1 = pp_o1.tile([P, h], F32, tag="op1")
                op2 = pp_o2.tile([P, a - h + 2], F32, tag="op2")
                for t in range(i + 1):
                    k0t, klt = s_tiles[t]
                    lhs = expT[t][:klt, q0 - k0t:q0 - k0t + il]
                    vt = v16_t0 if i == 0 else v_tiles[t]
                    nc.tensor.matmul(
                        op1[:il, :], lhs, vt[:klt, 0:h],
                        start=(t == 0), stop=(t == i),
                    )
                    nc.tensor.matmul(
                        op2[:il, :], lhs, vt[:klt, h:a + 2],
                        start=(t == 0), stop=(t == i),
                    )
                rec = pool.tile([P, 1], F32, tag="rec", bufs=2)
                nc.vector.reciprocal(rec[:il, :], op2[:il, a - h:a - h + 1])
                o_sb = pool.tile([P, a], F32, tag="o_sb", bufs=3)
                # epilogue split per half so scale/bias-add/store pipeline
                nc.vector.tensor_scalar_mul(
                    o_sb[:il, 0:h], op1[:il, :], rec[:il, 0:1])
                nc.gpsimd.tensor_add(
                    o_sb[:il, 0:h], o_sb[:il, 0:h], bv_bc[:il, 0:h])
                nc.sync.dma_start(out[b, q0:q0 + il, 0:h], o_sb[:il, 0:h])
                nc.vector.tensor_scalar_mul(
                    o_sb[:il, h:a], op2[:il, 0:a - h], rec[:il, 0:1])
                nc.gpsimd.tensor_add(
                    o_sb[:il, h:a], o_sb[:il, h:a], bv_bc[:il, h:a])
                nc.sync.dma_start(out[b, q0:q0 + il, h:a], o_sb[:il, h:a])

        # Plain per-batch emission (A, B, C+D). Hoisting the next batch's
        # stage A earlier in the engine FIFOs was tried and regresses:
        # hoisted transposes head-of-line block scores/PV whenever their
        # x DMA isn't done yet.
        for b in range(b_pc):
            st = stage_a_alloc(b)
            stage_a(b, *st, s_tiles)
            proj = stage_b(b, *st)
            stage_cd(b, *proj)

    nc.compile()
    return nc


_BUILT = {}


def _get_nc(b_pc, s, e, a):
    key = (b_pc, s, e, a)
    if key not in _BUILT:
        _BUILT[key] = build(b_pc, s, e, a)
    return _BUILT[key]


def _interleave_w(W, e, a):
    """Host-side DoubleRowSwInterleave weight prep: [j, m, K, 256] blocks,
    per row [A127, B127, ..., A0, B0] (A = E-rows 2j*128.., B = next 128,
    columns reversed), pre-scaled by W8_SCALE."""
    n_j, n_a = e // (2 * P), a // P
    out = np.empty((n_j, n_a, P, 2 * P), np.float32)
    Ws = np.asarray(W, np.float32) * W8_SCALE
    for j in range(n_j):
        A_ = Ws[2 * j * P:(2 * j + 1) * P]
        B_ = Ws[(2 * j + 1) * P:(2 * j + 2) * P]
        for m in range(n_a):
            out[j, m, :, 0::2] = A_[:, m * P:(m + 1) * P][:, ::-1]
            out[j, m, :, 1::2] = B_[:, m * P:(m + 1) * P][:, ::-1]
    return np.ascontiguousarray(out)


def run_sharded(inputs, b_pc, s, e, a, **run_kwargs):
    """Run the SPMD kernel over N_CORES cores, sharding batch dim of x."""
    x = np.ascontiguousarray(inputs["x"], dtype=np.float32)
    b_total = x.shape[0]
    assert b_total == b_pc * N_CORES
    shared = {
        "Wq": _interleave_w(inputs["Wq"], e, a),
        "Wk": _interleave_w(inputs["Wk"], e, a),
        "Wv": np.ascontiguousarray(inputs["Wv"], dtype=np.float32),
        "bq": np.ascontiguousarray(inputs["bq"], dtype=np.float32),
        "bk": np.ascontiguousarray(inputs["bk"], dtype=np.float32),
        "bv": np.ascontiguousarray(inputs["bv"], dtype=np.float32),
    }
    in_maps = [
        {"x": x[c * b_pc:(c + 1) * b_pc], **shared} for c in range(N_CORES)
    ]
    nc = _get_nc(b_pc, s, e, a)
    res = run_bass_kernel_spmd(nc, in_maps, core_ids=list(range(N_CORES)),
                               **run_kwargs)
    full = np.concatenate([res.results[c]["out"] for c in range(N_CORES)], axis=0)
    return full, res


def kernel(**inputs) -> np.ndarray:
    out, _ = run_sharded(inputs, B // N_CORES, S, E, A)
    return out
